# revision 6
# baseline (speedup 1.0000x reference)
"""Causal self-attention (GQA + RoPE) Trainium2 Bass kernel, 8 NeuronCores.

Sharding: 2-way data parallel over batch x 4-way tensor parallel over heads.
Core c handles batch c//4 and query heads [4*(c%4), 4*(c%4)+4) plus the one
KV head g = c%4 that serves them (n_kv_heads=4 -> no KV replication).
Each core computes a partial [S, D] output (its heads' slice of the out
projection); the host sums the 4 partials per batch.

Device layouts are transposed ("feature-major"): x is transposed on-chip via
PE transposes; projections produce qT/kT/vT [dim, tokens]; attention scores
are computed as S^T = kT.T @ qT so softmax denominators come from a
ones-vector matmul (partition-dim sum) and the P@V contraction needs no
transposed probabilities.  RoPE is handled by de-interleaving the q/k weight
rows on the host so the rotation pairs become (p, p+64) partition pairs.
TensorEngine-facing tensors are bf16 (fp32 PSUM accumulation); softmax
tables/masks and the output stay fp32.
"""

import sys

if "/opt/trn_rl_repo" not in sys.path:
    sys.path.insert(0, "/opt/trn_rl_repo")

import math

import numpy as np

D_MODEL = 2048
N_HEADS = 16
N_KV_HEADS = 4
ROPE_THETA = 10000.0
B, S = 2, 2048
DK = D_MODEL // N_HEADS          # 128
NCORES = 8
NEG = -1e30

_COMPILED = None
_TRACE = False                   # test.py flips this for profiling runs
_LAST_RESULT = None              # BassKernelResults of the last run


def _build():
    import concourse.bacc as bacc
    import concourse.tile as tile
    from concourse import mybir

    f32 = mybir.dt.float32
    bf16 = mybir.dt.bfloat16

    nc = bacc.Bacc("TRN2", debug=False, target_bir_lowering=False)

    def inp(name, shape, dt=bf16):
        return nc.declare_dram_parameter(name, list(shape), dt, isOutput=False).ap()

    x_d = inp("x", [S, D_MODEL])
    wq_d = inp("wq", [128, 16, 512])
    wkv_d = inp("wkv", [128, 16, 256])
    wc_d = inp("wc", [128, 4, 2048])
    cos_d = inp("cos2", [64, S], f32)
    sin_d = inp("ss", [64, S], f32)
    dmask_d = inp("dmask", [128, 128], f32)
    ident_d = inp("ident", [128, 128])
    onescol_d = inp("onescol", [128, 1])
    onesrow_d = inp("onesrow", [1, 128])
    out_d = nc.declare_dram_parameter("out", [S, D_MODEL], f32, isOutput=True).ap()

    EXP = mybir.ActivationFunctionType.Exp

    with tile.TileContext(nc) as tc:
        with (
            tc.tile_pool(name="consts", bufs=1) as consts,
            tc.tile_pool(name="xpool", bufs=2) as xpool,
            tc.tile_pool(name="xt", bufs=1) as xtp,
            tc.tile_pool(name="qpool", bufs=2) as qpool,
            tc.tile_pool(name="vch", bufs=2) as vchp,
            tc.tile_pool(name="tmp", bufs=2) as tmpp,
            tc.tile_pool(name="epool", bufs=4) as epool,
            tc.tile_pool(name="rsum", bufs=2) as rsp,
            tc.tile_pool(name="otp", bufs=2) as otp,
            tc.tile_pool(name="osb", bufs=3) as osbp,
            tc.tile_pool(name="psum", bufs=1, space="PSUM") as psum,
        ):
            # ---- constants / weights ----
            wq_sb = consts.tile([128, 16, 512], bf16, tag="wq")
            wkv_sb = consts.tile([128, 16, 256], bf16, tag="wkv")
            wc_sb = consts.tile([128, 4, 2048], bf16, tag="wc")
            c2_sb = consts.tile([64, S], f32, tag="cos2")
            ss_sb = consts.tile([64, S], f32, tag="ss")
            dmask_sb = consts.tile([128, 128], f32, tag="dmask")
            ident_sb = consts.tile([128, 128], bf16, tag="ident")
            onescol_sb = consts.tile([128, 1], bf16, tag="onescol")
            onesrow_sb = consts.tile([1, 128], bf16, tag="onesrow")
            kTr_sb = consts.tile([128, S], bf16, tag="kTr")
            v_sb = consts.tile([128, 16, 128], bf16, tag="V")

            nc.sync.dma_start(out=wq_sb, in_=wq_d)
            nc.sync.dma_start(out=wkv_sb, in_=wkv_d)
            nc.sync.dma_start(out=wc_sb, in_=wc_d)
            nc.sync.dma_start(out=c2_sb, in_=cos_d)
            nc.sync.dma_start(out=ss_sb, in_=sin_d)
            nc.sync.dma_start(out=dmask_sb, in_=dmask_d)
            nc.sync.dma_start(out=ident_sb, in_=ident_d)
            nc.sync.dma_start(out=onescol_sb, in_=onescol_d)
            nc.sync.dma_start(out=onesrow_sb, in_=onesrow_d)

            def rope(dst, src, c):
                """dst[128,512] (bf16 SBUF) <- rotate(src[128,512] f32 PSUM).

                Row p<64 holds the even (te) element of pair p, row p+64 the
                odd (to): dst_lo = te*cos - to*sin; dst_hi = to*cos + te*sin.
                """
                cs = c2_sb[:, c * 512:(c + 1) * 512]
                sn = ss_sb[:, c * 512:(c + 1) * 512]
                t = tmpp.tile([128, 512], f32, tag="ropesin")
                t2 = tmpp.tile([128, 512], f32, tag="ropecos")
                nc.vector.tensor_mul(t[0:64, :], src[64:128, :], sn)
                nc.vector.tensor_mul(t[64:128, :], src[0:64, :], sn)
                nc.vector.tensor_mul(t2[0:64, :], src[0:64, :], cs)
                nc.vector.tensor_mul(t2[64:128, :], src[64:128, :], cs)
                nc.vector.tensor_sub(dst[0:64, :], t2[0:64, :], t[0:64, :])
                nc.vector.tensor_add(dst[64:128, :], t2[64:128, :], t[64:128, :])

            for c in range(4):  # 512-token chunks
                tq0 = c * 512
                # ---- transpose x chunk -> xT [128d, 16db, 512t] ----
                xT = xtp.tile([128, 16, 512], bf16, tag="xT")
                for tb in range(4):
                    row = tq0 + tb * 128
                    for half in range(2):
                        xsb = xpool.tile([128, 1024], bf16, tag="x")
                        nc.sync.dma_start(
                            out=xsb,
                            in_=x_d[row:row + 128, half * 1024:(half + 1) * 1024],
                        )
                        for dd in range(8):
                            db = half * 8 + dd
                            pxt = psum.tile([128, 128], bf16, tag="xt")
                            nc.tensor.transpose(
                                pxt, xsb[:, dd * 128:(dd + 1) * 128], ident_sb
                            )
                            nc.scalar.copy(
                                out=xT[:, db, tb * 128:(tb + 1) * 128], in_=pxt
                            )

                # ---- Q projection + rope ----
                qTr = qpool.tile([128, 4, 512], bf16, tag="qTr")
                for m in range(4):
                    pq = psum.tile([128, 512], f32, tag="mm512")
                    for db in range(16):
                        nc.tensor.matmul(
                            pq,
                            lhsT=wq_sb[:, db, m * 128:(m + 1) * 128],
                            rhs=xT[:, db, :],
                            start=(db == 0),
                            stop=(db == 15),
                        )
                    rope(qTr[:, m, :], pq, c)

                # ---- K projection + rope ----
                pk = psum.tile([128, 512], f32, tag="mm512")
                for db in range(16):
                    nc.tensor.matmul(
                        pk,
                        lhsT=wkv_sb[:, db, 0:128],
                        rhs=xT[:, db, :],
                        start=(db == 0),
                        stop=(db == 15),
                    )
                rope(kTr_sb[:, tq0:tq0 + 512], pk, c)

                # ---- V projection; transpose to [tk, hd] tiles ----
                pv = psum.tile([128, 512], f32, tag="mm512")
                for db in range(16):
                    nc.tensor.matmul(
                        pv,
                        lhsT=wkv_sb[:, db, 128:256],
                        rhs=xT[:, db, :],
                        start=(db == 0),
                        stop=(db == 15),
                    )
                vch = vchp.tile([128, 512], bf16, tag="vch")
                nc.scalar.copy(out=vch, in_=pv)
                for rr in range(4):
                    pvt = psum.tile([128, 128], bf16, tag="xt")
                    nc.tensor.transpose(
                        pvt, vch[:, rr * 128:(rr + 1) * 128], ident_sb
                    )
                    nc.scalar.copy(out=v_sb[:, 4 * c + rr, :], in_=pvt)

                # ---- attention for tq chunk c, all 4 heads ----
                nkb = 4 * c + 4
                otc = otp.tile([128, 4, 512], bf16, tag="OT")
                for h in range(4):
                    psum_sum = psum.tile([1, 512], f32, tag="sums")
                    psum_ot = psum.tile([128, 512], f32, tag="ot")
                    for kb in range(nkb):
                        rr = kb - 4 * c  # >= 0 on the diagonal chunk group
                        col0 = 0 if rr < 0 else 128 * rr
                        pst = psum.tile([128, 512], f32, tag="st")
                        nc.tensor.matmul(
                            pst[:, col0:512],
                            lhsT=kTr_sb[:, kb * 128:(kb + 1) * 128],
                            rhs=qTr[:, h, col0:512],
                            start=True,
                            stop=True,
                        )
                        if rr >= 0:
                            nc.vector.tensor_add(
                                pst[:, col0:col0 + 128],
                                pst[:, col0:col0 + 128],
                                dmask_sb,
                            )
                        e = epool.tile([128, 512], bf16, tag="E")
                        if col0 > 0:
                            nc.vector.memset(e[:, 0:col0], 0.0)
                        nc.scalar.activation(
                            out=e[:, col0:512], in_=pst[:, col0:512], func=EXP
                        )
                        nc.tensor.matmul(
                            psum_sum,
                            lhsT=onescol_sb,
                            rhs=e,
                            start=(kb == 0),
                            stop=(kb == nkb - 1),
                        )
                        nc.tensor.matmul(
                            psum_ot,
                            lhsT=v_sb[:, kb, :],
                            rhs=e,
                            start=(kb == 0),
                            stop=(kb == nkb - 1),
                        )
                    rsum = rsp.tile([1, 512], f32, tag="rsum")
                    rsumb = rsp.tile([1, 512], bf16, tag="rsumb")
                    nc.vector.reciprocal(out=rsum, in_=psum_sum)
                    nc.vector.tensor_copy(out=rsumb, in_=rsum)
                    pb = psum.tile([128, 512], f32, tag="st")
                    nc.tensor.matmul(
                        pb, lhsT=onesrow_sb, rhs=rsumb, start=True, stop=True
                    )
                    nc.scalar.copy(out=otc[:, h, :], in_=psum_ot)
                    nc.vector.tensor_mul(otc[:, h, :], otc[:, h, :], pb)

                # ---- output projection for this chunk's rows ----
                for tb in range(4):
                    row = tq0 + tb * 128
                    for oc in range(4):
                        po = psum.tile([128, 512], f32, tag="mm512")
                        for h in range(4):
                            nc.tensor.matmul(
                                po,
                                lhsT=otc[:, h, tb * 128:(tb + 1) * 128],
                                rhs=wc_sb[:, h, oc * 512:(oc + 1) * 512],
                                start=(h == 0),
                                stop=(h == 3),
                            )
                        osb = osbp.tile([128, 512], f32, tag="osb")
                        nc.vector.tensor_copy(out=osb, in_=po)
                        nc.sync.dma_start(
                            out=out_d[row:row + 128, oc * 512:(oc + 1) * 512],
                            in_=osb,
                        )

    nc.compile()
    return nc


def _host_prep(x, Wq, Wkv, Wc):
    """Shard + relayout the full inputs into the 8 per-core input dicts."""
    import ml_dtypes

    bf = ml_dtypes.bfloat16
    dk, H, KV = DK, N_HEADS, N_KV_HEADS
    x = np.asarray(x, np.float32)
    Wq = np.asarray(Wq, np.float32)
    Wkv = np.asarray(Wkv, np.float32)
    Wc = np.asarray(Wc, np.float32)

    p = np.concatenate([np.arange(0, dk, 2), np.arange(1, dk, 2)])
    perm_q = np.concatenate([h * dk + p for h in range(H)])
    Wq_p = (Wq / math.sqrt(dk))[perm_q]
    perm_k = np.concatenate([g * dk + p for g in range(KV)])
    Wk_p = Wkv[:KV * dk][perm_k]
    Wv = Wkv[KV * dk:]

    pairs = np.arange(dk // 2, dtype=np.float64)
    freqs = 1.0 / (ROPE_THETA ** (2.0 * pairs / dk))
    ang = np.arange(S, dtype=np.float64)[:, None] * freqs[None, :]
    c2 = np.ascontiguousarray(np.cos(ang).astype(np.float32).T)  # [64, S]
    ss = np.ascontiguousarray(np.sin(ang).astype(np.float32).T)

    jj = np.arange(128)[None, :]
    pp = np.arange(128)[:, None]
    dmask = np.where(pp <= jj, 0.0, NEG).astype(np.float32)
    ident = np.eye(128, dtype=bf)
    onescol = np.ones((128, 1), bf)
    onesrow = np.ones((1, 128), bf)

    maps = []
    for core in range(NCORES):
        b, g = core // 4, core % 4
        wq_l = np.ascontiguousarray(
            Wq_p[512 * g:512 * g + 512].T.reshape(16, 128, 512).transpose(1, 0, 2)
        ).astype(bf)
        wkv_sl = np.concatenate(
            [Wk_p[g * dk:(g + 1) * dk], Wv[g * dk:(g + 1) * dk]], 0
        ).T  # [2048, 256]
        wkv_l = np.ascontiguousarray(
            wkv_sl.reshape(16, 128, 256).transpose(1, 0, 2)
        ).astype(bf)
        wc_l = np.ascontiguousarray(
            Wc[:, 512 * g:512 * g + 512].T.reshape(4, 128, 2048).transpose(1, 0, 2)
        ).astype(bf)
        maps.append(dict(
            x=np.ascontiguousarray(x[b]).astype(bf), wq=wq_l, wkv=wkv_l, wc=wc_l,
            cos2=c2, ss=ss, dmask=dmask, ident=ident,
            onescol=onescol, onesrow=onesrow,
        ))
    return maps


def kernel(x, Wq, Wkv, Wc):
    global _COMPILED, _LAST_RESULT
    from concourse.bass_utils import run_bass_kernel_spmd

    if _COMPILED is None:
        _COMPILED = _build()
    in_maps = _host_prep(x, Wq, Wkv, Wc)
    res = run_bass_kernel_spmd(
        _COMPILED, in_maps, core_ids=list(range(NCORES)), trace=_TRACE
    )
    _LAST_RESULT = res
    outs = [res.results[i]["out"] for i in range(NCORES)]
    full = np.stack(
        [outs[0] + outs[1] + outs[2] + outs[3],
         outs[4] + outs[5] + outs[6] + outs[7]], 0
    ).astype(np.float32)
    return full


# revision 7
# speedup vs baseline: 1.6741x; 1.6741x over previous
"""Causal self-attention (GQA + RoPE) Trainium2 Bass kernel, 8 NeuronCores.

Sharding: 2-way data parallel over batch x 4-way tensor parallel over heads.
Core c handles batch c//4 and query heads [4*(c%4), 4*(c%4)+4) plus the one
KV head g = c%4 that serves them (n_kv_heads=4 -> no KV replication).
Each core computes a partial [S, D] output (its heads' slice of the out
projection); the host sums the 4 partials per batch.

Device layouts are transposed ("feature-major"): x is transposed on-chip via
PE transposes; projections produce qT/kT/vT [dim, tokens]; attention scores
are computed as S^T = kT.T @ qT so softmax denominators come from a
ones-vector matmul (partition-dim sum) and the P@V contraction needs no
transposed probabilities.  RoPE is handled by de-interleaving the q/k weight
rows on the host so the rotation pairs become (p, p+64) partition pairs.
TensorEngine-facing tensors are bf16 (fp32 PSUM accumulation); softmax
tables/masks and the output stay fp32.
"""

import sys

if "/opt/trn_rl_repo" not in sys.path:
    sys.path.insert(0, "/opt/trn_rl_repo")

import math

import numpy as np

D_MODEL = 2048
N_HEADS = 16
N_KV_HEADS = 4
ROPE_THETA = 10000.0
B, S = 2, 2048
DK = D_MODEL // N_HEADS          # 128
NCORES = 8
NEG = -1e30

_COMPILED = None
_TRACE = False                   # test.py flips this for profiling runs
_LAST_RESULT = None              # BassKernelResults of the last run


def _build():
    import concourse.bacc as bacc
    import concourse.tile as tile
    from concourse import mybir

    f32 = mybir.dt.float32
    bf16 = mybir.dt.bfloat16

    nc = bacc.Bacc("TRN2", debug=False, target_bir_lowering=False)

    def inp(name, shape, dt=bf16):
        return nc.declare_dram_parameter(name, list(shape), dt, isOutput=False).ap()

    x_d = inp("x", [S, D_MODEL])
    wq_d = inp("wq", [128, 16, 512])
    wkv_d = inp("wkv", [128, 16, 256])
    wc_d = inp("wc", [128, 4, 2048])
    cos_d = inp("cos2", [64, S], f32)
    sin_d = inp("ss", [64, S], f32)
    dmask_d = inp("dmask", [128, 128], f32)
    ident_d = inp("ident", [128, 128])
    onescol_d = inp("onescol", [128, 1])
    onesrow_d = inp("onesrow", [1, 128])
    out_d = nc.declare_dram_parameter("out", [S, D_MODEL], f32, isOutput=True).ap()

    EXP = mybir.ActivationFunctionType.Exp
    LN = mybir.ActivationFunctionType.Ln

    with tile.TileContext(nc) as tc:
        with (
            tc.tile_pool(name="consts", bufs=1) as consts,
            tc.tile_pool(name="xt", bufs=2) as xtp,
            tc.tile_pool(name="qpool", bufs=2) as qpool,
            tc.tile_pool(name="vch", bufs=2) as vchp,
            tc.tile_pool(name="tmp", bufs=2) as tmpp,
            tc.tile_pool(name="epool", bufs=6) as epool,
            tc.tile_pool(name="rsum", bufs=2) as rsp,
            tc.tile_pool(name="otp", bufs=2) as otp,
            tc.tile_pool(name="osb", bufs=3) as osbp,
            tc.tile_pool(name="psum", bufs=2, space="PSUM") as psum,
            tc.tile_pool(name="psum_st", bufs=3, space="PSUM") as psum_st,
            tc.tile_pool(name="psum_acc", bufs=1, space="PSUM") as psum_acc,
        ):
            # ---- constants / weights ----
            wq_sb = consts.tile([128, 16, 512], bf16, tag="wq")
            wkv_sb = consts.tile([128, 16, 256], bf16, tag="wkv")
            wc_sb = consts.tile([128, 4, 2048], bf16, tag="wc")
            c2_sb = consts.tile([64, S], f32, tag="cos2")
            ss_sb = consts.tile([64, S], f32, tag="ss")
            dmask_sb = consts.tile([128, 128], f32, tag="dmask")
            ident_sb = consts.tile([128, 128], bf16, tag="ident")
            onescol_sb = consts.tile([128, 1], bf16, tag="onescol")
            onesrow_sb = consts.tile([1, 128], bf16, tag="onesrow")
            kTr_sb = consts.tile([128, S], bf16, tag="kTr")
            v_sb = consts.tile([128, 16, 128], bf16, tag="V")

            nc.sync.dma_start(out=wq_sb, in_=wq_d)
            nc.sync.dma_start(out=wkv_sb, in_=wkv_d)
            nc.sync.dma_start(out=wc_sb, in_=wc_d)
            nc.sync.dma_start(out=c2_sb, in_=cos_d)
            nc.sync.dma_start(out=ss_sb, in_=sin_d)
            nc.sync.dma_start(out=dmask_sb, in_=dmask_d)
            nc.sync.dma_start(out=ident_sb, in_=ident_d)
            nc.sync.dma_start(out=onescol_sb, in_=onescol_d)
            nc.sync.dma_start(out=onesrow_sb, in_=onesrow_d)

            def rope(dst, src, c):
                """dst[128,512] (bf16 SBUF) <- rotate(src[128,512] f32 PSUM).

                Row p<64 holds the even (te) element of pair p, row p+64 the
                odd (to): dst_lo = te*cos - to*sin; dst_hi = to*cos + te*sin.
                """
                cs = c2_sb[:, c * 512:(c + 1) * 512]
                sn = ss_sb[:, c * 512:(c + 1) * 512]
                t = tmpp.tile([128, 512], f32, tag="ropesin")
                t2 = tmpp.tile([128, 512], f32, tag="ropecos")
                nc.vector.tensor_mul(t[0:64, :], src[64:128, :], sn)
                nc.vector.tensor_mul(t[64:128, :], src[0:64, :], sn)
                nc.vector.tensor_mul(t2[0:64, :], src[0:64, :], cs)
                nc.vector.tensor_mul(t2[64:128, :], src[64:128, :], cs)
                nc.vector.tensor_sub(dst[0:64, :], t2[0:64, :], t[0:64, :])
                nc.vector.tensor_add(dst[64:128, :], t2[64:128, :], t[64:128, :])

            for c in range(4):  # 512-token chunks
                tq0 = c * 512
                # ---- transpose x chunk -> xT [128d, 16db, 512t] (DMA xbar) ----
                xT = xtp.tile([128, 16, 512], bf16, tag="xT")
                for db in range(16):
                    nc.sync.dma_start_transpose(
                        out=xT[:, db, :],
                        in_=x_d[tq0:tq0 + 512, db * 128:(db + 1) * 128],
                    )

                # ---- Q projection + rope ----
                qTr = qpool.tile([128, 4, 512], bf16, tag="qTr")
                for m in range(4):
                    pq = psum.tile([128, 512], f32, tag="mm512")
                    for db in range(16):
                        nc.tensor.matmul(
                            pq,
                            lhsT=wq_sb[:, db, m * 128:(m + 1) * 128],
                            rhs=xT[:, db, :],
                            start=(db == 0),
                            stop=(db == 15),
                        )
                    rope(qTr[:, m, :], pq, c)

                # ---- K projection + rope ----
                pk = psum.tile([128, 512], f32, tag="mm512")
                for db in range(16):
                    nc.tensor.matmul(
                        pk,
                        lhsT=wkv_sb[:, db, 0:128],
                        rhs=xT[:, db, :],
                        start=(db == 0),
                        stop=(db == 15),
                    )
                rope(kTr_sb[:, tq0:tq0 + 512], pk, c)

                # ---- V projection; transpose to [tk, hd] tiles ----
                pv = psum.tile([128, 512], f32, tag="mm512")
                for db in range(16):
                    nc.tensor.matmul(
                        pv,
                        lhsT=wkv_sb[:, db, 128:256],
                        rhs=xT[:, db, :],
                        start=(db == 0),
                        stop=(db == 15),
                    )
                vch = vchp.tile([128, 512], bf16, tag="vch")
                nc.scalar.copy(out=vch, in_=pv)
                for rr in range(4):
                    nc.sync.dma_start_transpose(
                        out=v_sb[:, 4 * c + rr, :],
                        in_=vch[:, rr * 128:(rr + 1) * 128],
                    )

                # ---- attention for tq chunk c, all 4 heads ----
                nkb = 4 * c + 4
                otc = otp.tile([128, 4, 512], bf16, tag="OT")
                for h in range(4):
                    psum_sum = psum_acc.tile([1, 512], f32, tag="sums")
                    psum_ot = psum.tile([128, 512], f32, tag="ot")
                    for kb in range(nkb):
                        rr = kb - 4 * c  # >= 0 on the diagonal chunk group
                        col0 = 0 if rr < 0 else 128 * rr
                        pst = psum_st.tile([128, 512], f32, tag="st")
                        nc.tensor.matmul(
                            pst[:, col0:512],
                            lhsT=kTr_sb[:, kb * 128:(kb + 1) * 128],
                            rhs=qTr[:, h, col0:512],
                            start=True,
                            stop=True,
                        )
                        if rr >= 0:
                            nc.vector.tensor_add(
                                pst[:, col0:col0 + 128],
                                pst[:, col0:col0 + 128],
                                dmask_sb,
                            )
                        e = epool.tile([128, 512], bf16, tag="E")
                        if col0 > 0:
                            nc.vector.memset(e[:, 0:col0], 0.0)
                        nc.scalar.activation(
                            out=e[:, col0:512], in_=pst[:, col0:512], func=EXP
                        )
                        nc.tensor.matmul(
                            psum_sum,
                            lhsT=onescol_sb,
                            rhs=e,
                            start=(kb == 0),
                            stop=(kb == nkb - 1),
                        )
                        nc.tensor.matmul(
                            psum_ot,
                            lhsT=v_sb[:, kb, :],
                            rhs=e,
                            start=(kb == 0),
                            stop=(kb == nkb - 1),
                        )
                    lns = rsp.tile([1, 512], f32, tag="lns")
                    rsumb = rsp.tile([1, 512], bf16, tag="rsumb")
                    nc.scalar.activation(out=lns, in_=psum_sum, func=LN)
                    nc.scalar.activation(out=rsumb, in_=lns, func=EXP, scale=-1.0)
                    pb = psum_st.tile([128, 512], f32, tag="st")
                    nc.tensor.matmul(
                        pb, lhsT=onesrow_sb, rhs=rsumb, start=True, stop=True
                    )
                    nc.scalar.copy(out=otc[:, h, :], in_=psum_ot)
                    nc.vector.tensor_mul(otc[:, h, :], otc[:, h, :], pb)

                # ---- output projection for this chunk's rows ----
                for tb in range(4):
                    row = tq0 + tb * 128
                    for oc in range(4):
                        po = psum.tile([128, 512], f32, tag="mm512")
                        for h in range(4):
                            nc.tensor.matmul(
                                po,
                                lhsT=otc[:, h, tb * 128:(tb + 1) * 128],
                                rhs=wc_sb[:, h, oc * 512:(oc + 1) * 512],
                                start=(h == 0),
                                stop=(h == 3),
                            )
                        osb = osbp.tile([128, 512], f32, tag="osb")
                        nc.scalar.copy(out=osb, in_=po)
                        nc.sync.dma_start(
                            out=out_d[row:row + 128, oc * 512:(oc + 1) * 512],
                            in_=osb,
                        )

    nc.compile()
    return nc


def _host_prep(x, Wq, Wkv, Wc):
    """Shard + relayout the full inputs into the 8 per-core input dicts."""
    import ml_dtypes

    bf = ml_dtypes.bfloat16
    dk, H, KV = DK, N_HEADS, N_KV_HEADS
    x = np.asarray(x, np.float32)
    Wq = np.asarray(Wq, np.float32)
    Wkv = np.asarray(Wkv, np.float32)
    Wc = np.asarray(Wc, np.float32)

    p = np.concatenate([np.arange(0, dk, 2), np.arange(1, dk, 2)])
    perm_q = np.concatenate([h * dk + p for h in range(H)])
    Wq_p = (Wq / math.sqrt(dk))[perm_q]
    perm_k = np.concatenate([g * dk + p for g in range(KV)])
    Wk_p = Wkv[:KV * dk][perm_k]
    Wv = Wkv[KV * dk:]

    pairs = np.arange(dk // 2, dtype=np.float64)
    freqs = 1.0 / (ROPE_THETA ** (2.0 * pairs / dk))
    ang = np.arange(S, dtype=np.float64)[:, None] * freqs[None, :]
    c2 = np.ascontiguousarray(np.cos(ang).astype(np.float32).T)  # [64, S]
    ss = np.ascontiguousarray(np.sin(ang).astype(np.float32).T)

    jj = np.arange(128)[None, :]
    pp = np.arange(128)[:, None]
    dmask = np.where(pp <= jj, 0.0, NEG).astype(np.float32)
    ident = np.eye(128, dtype=bf)
    onescol = np.ones((128, 1), bf)
    onesrow = np.ones((1, 128), bf)

    maps = []
    for core in range(NCORES):
        b, g = core // 4, core % 4
        wq_l = np.ascontiguousarray(
            Wq_p[512 * g:512 * g + 512].T.reshape(16, 128, 512).transpose(1, 0, 2)
        ).astype(bf)
        wkv_sl = np.concatenate(
            [Wk_p[g * dk:(g + 1) * dk], Wv[g * dk:(g + 1) * dk]], 0
        ).T  # [2048, 256]
        wkv_l = np.ascontiguousarray(
            wkv_sl.reshape(16, 128, 256).transpose(1, 0, 2)
        ).astype(bf)
        wc_l = np.ascontiguousarray(
            Wc[:, 512 * g:512 * g + 512].T.reshape(4, 128, 2048).transpose(1, 0, 2)
        ).astype(bf)
        maps.append(dict(
            x=np.ascontiguousarray(x[b]).astype(bf), wq=wq_l, wkv=wkv_l, wc=wc_l,
            cos2=c2, ss=ss, dmask=dmask, ident=ident,
            onescol=onescol, onesrow=onesrow,
        ))
    return maps


def kernel(x, Wq, Wkv, Wc):
    global _COMPILED, _LAST_RESULT
    from concourse.bass_utils import run_bass_kernel_spmd

    if _COMPILED is None:
        _COMPILED = _build()
    in_maps = _host_prep(x, Wq, Wkv, Wc)
    res = run_bass_kernel_spmd(
        _COMPILED, in_maps, core_ids=list(range(NCORES)), trace=_TRACE
    )
    _LAST_RESULT = res
    outs = [res.results[i]["out"] for i in range(NCORES)]
    full = np.stack(
        [outs[0] + outs[1] + outs[2] + outs[3],
         outs[4] + outs[5] + outs[6] + outs[7]], 0
    ).astype(np.float32)
    return full


# revision 8
# speedup vs baseline: 1.8197x; 1.0870x over previous
"""Causal self-attention (GQA + RoPE) Trainium2 Bass kernel, 8 NeuronCores.

Sharding: 2-way data parallel over batch x 4-way tensor parallel over heads.
Core c handles batch c//4 and query heads [4*(c%4), 4*(c%4)+4) plus the one
KV head g = c%4 that serves them (n_kv_heads=4 -> no KV replication).
Each core computes a partial [S, D] output (its heads' slice of the out
projection); the host sums the 4 partials per batch.

Device layouts are transposed ("feature-major"): x is transposed on-chip via
PE transposes; projections produce qT/kT/vT [dim, tokens]; attention scores
are computed as S^T = kT.T @ qT so softmax denominators come from a
ones-vector matmul (partition-dim sum) and the P@V contraction needs no
transposed probabilities.  RoPE is handled by de-interleaving the q/k weight
rows on the host so the rotation pairs become (p, p+64) partition pairs.
TensorEngine-facing tensors are bf16 (fp32 PSUM accumulation); softmax
tables/masks and the output stay fp32.
"""

import sys

if "/opt/trn_rl_repo" not in sys.path:
    sys.path.insert(0, "/opt/trn_rl_repo")

import math

import numpy as np

D_MODEL = 2048
N_HEADS = 16
N_KV_HEADS = 4
ROPE_THETA = 10000.0
B, S = 2, 2048
DK = D_MODEL // N_HEADS          # 128
NCORES = 8
NEG = -1e30

_COMPILED = None
_TRACE = False                   # test.py flips this for profiling runs
_LAST_RESULT = None              # BassKernelResults of the last run


def _build():
    import concourse.bacc as bacc
    import concourse.tile as tile
    from concourse import mybir

    f32 = mybir.dt.float32
    bf16 = mybir.dt.bfloat16

    nc = bacc.Bacc("TRN2", debug=False, target_bir_lowering=False)

    def inp(name, shape, dt=bf16):
        return nc.declare_dram_parameter(name, list(shape), dt, isOutput=False).ap()

    x_d = inp("x", [S, D_MODEL])
    wq_d = inp("wq", [128, 16, 512])
    wkv_d = inp("wkv", [128, 16, 256])
    wc_d = inp("wc", [128, 4, 2048])
    cos_d = inp("cos2", [64, S], f32)
    sin_d = inp("ss", [64, S], f32)
    dmask_d = inp("dmask", [128, 128], f32)
    ident_d = inp("ident", [128, 128])
    onescol_d = inp("onescol", [128, 1])
    onesrow_d = inp("onesrow", [1, 128])
    out_d = nc.declare_dram_parameter("out", [S, D_MODEL], f32, isOutput=True).ap()

    EXP = mybir.ActivationFunctionType.Exp
    LN = mybir.ActivationFunctionType.Ln

    with tile.TileContext(nc) as tc:
        with (
            tc.tile_pool(name="consts", bufs=1) as consts,
            tc.tile_pool(name="xt", bufs=4) as xtp,
            tc.tile_pool(name="qpool", bufs=2) as qpool,
            tc.tile_pool(name="vch", bufs=2) as vchp,
            tc.tile_pool(name="tmp", bufs=2) as tmpp,
            tc.tile_pool(name="epool", bufs=6) as epool,
            tc.tile_pool(name="rsum", bufs=2) as rsp,
            tc.tile_pool(name="otp", bufs=2) as otp,
            tc.tile_pool(name="osb", bufs=3) as osbp,
            tc.tile_pool(name="psum", bufs=2, space="PSUM") as psum,
            tc.tile_pool(name="psum_st", bufs=2, space="PSUM") as psum_st,
            tc.tile_pool(name="psum_ot", bufs=2, space="PSUM") as psum_otp,
            tc.tile_pool(name="psum_acc", bufs=1, space="PSUM") as psum_acc,
            tc.tile_pool(name="psum_pb", bufs=1, space="PSUM") as psum_pb,
        ):
            # ---- constants / weights ----
            wq_sb = consts.tile([128, 16, 512], bf16, tag="wq")
            wkv_sb = consts.tile([128, 16, 256], bf16, tag="wkv")
            wc_sb = consts.tile([128, 4, 2048], bf16, tag="wc")
            c2_sb = consts.tile([64, S], f32, tag="cos2")
            ss_sb = consts.tile([64, S], f32, tag="ss")
            dmask_sb = consts.tile([128, 128], f32, tag="dmask")
            ident_sb = consts.tile([128, 128], bf16, tag="ident")
            onescol_sb = consts.tile([128, 1], bf16, tag="onescol")
            onesrow_sb = consts.tile([1, 128], bf16, tag="onesrow")
            kTr_sb = consts.tile([128, S], bf16, tag="kTr")
            v_sb = consts.tile([128, 16, 128], bf16, tag="V")

            nc.sync.dma_start(out=wq_sb, in_=wq_d)
            nc.sync.dma_start(out=wkv_sb, in_=wkv_d)
            nc.sync.dma_start(out=wc_sb, in_=wc_d)
            nc.sync.dma_start(out=c2_sb, in_=cos_d)
            nc.sync.dma_start(out=ss_sb, in_=sin_d)
            nc.sync.dma_start(out=dmask_sb, in_=dmask_d)
            nc.sync.dma_start(out=ident_sb, in_=ident_d)
            nc.sync.dma_start(out=onescol_sb, in_=onescol_d)
            nc.sync.dma_start(out=onesrow_sb, in_=onesrow_d)

            def rope(dst, src, c):
                """dst[128,512] (bf16 SBUF) <- rotate(src[128,512] f32 PSUM).

                Row p<64 holds the even (te) element of pair p, row p+64 the
                odd (to): dst_lo = te*cos - to*sin; dst_hi = to*cos + te*sin.
                """
                cs = c2_sb[:, c * 512:(c + 1) * 512]
                sn = ss_sb[:, c * 512:(c + 1) * 512]
                t = tmpp.tile([128, 512], f32, tag="ropesin")
                t2 = tmpp.tile([128, 512], f32, tag="ropecos")
                nc.vector.tensor_mul(t[0:64, :], src[64:128, :], sn)
                nc.vector.tensor_mul(t[64:128, :], src[0:64, :], sn)
                nc.vector.tensor_mul(t2[0:64, :], src[0:64, :], cs)
                nc.vector.tensor_mul(t2[64:128, :], src[64:128, :], cs)
                nc.vector.tensor_sub(dst[0:64, :], t2[0:64, :], t[0:64, :])
                nc.vector.tensor_add(dst[64:128, :], t2[64:128, :], t[64:128, :])

            for c in range(4):  # 512-token chunks
                tq0 = c * 512
                # ---- transpose x chunk -> xT [128d, 16db, 512t] (DMA xbar) ----
                xT = xtp.tile([128, 16, 512], bf16, tag="xT")
                for db in range(16):
                    nc.sync.dma_start_transpose(
                        out=xT[:, db, :],
                        in_=x_d[tq0:tq0 + 512, db * 128:(db + 1) * 128],
                    )

                # ---- Q projection + rope ----
                qTr = qpool.tile([128, 4, 512], bf16, tag="qTr")
                for m in range(4):
                    pq = psum.tile([128, 512], f32, tag="mm512")
                    for db in range(16):
                        nc.tensor.matmul(
                            pq,
                            lhsT=wq_sb[:, db, m * 128:(m + 1) * 128],
                            rhs=xT[:, db, :],
                            start=(db == 0),
                            stop=(db == 15),
                        )
                    rope(qTr[:, m, :], pq, c)

                # ---- K projection + rope ----
                pk = psum.tile([128, 512], f32, tag="mm512")
                for db in range(16):
                    nc.tensor.matmul(
                        pk,
                        lhsT=wkv_sb[:, db, 0:128],
                        rhs=xT[:, db, :],
                        start=(db == 0),
                        stop=(db == 15),
                    )
                rope(kTr_sb[:, tq0:tq0 + 512], pk, c)

                # ---- V projection; transpose to [tk, hd] tiles ----
                pv = psum.tile([128, 512], f32, tag="mm512")
                for db in range(16):
                    nc.tensor.matmul(
                        pv,
                        lhsT=wkv_sb[:, db, 128:256],
                        rhs=xT[:, db, :],
                        start=(db == 0),
                        stop=(db == 15),
                    )
                vch = vchp.tile([128, 512], bf16, tag="vch")
                nc.scalar.copy(out=vch, in_=pv)
                for rr in range(4):
                    nc.sync.dma_start_transpose(
                        out=v_sb[:, 4 * c + rr, :],
                        in_=vch[:, rr * 128:(rr + 1) * 128],
                    )

                # ---- attention for tq chunk c, all 4 heads ----
                nkb = 4 * c + 4
                otc = otp.tile([128, 4, 512], bf16, tag="OT")
                for h in range(4):
                    psum_sum = psum_acc.tile([1, 512], f32, tag="sums")
                    psum_ot = psum_otp.tile([128, 512], f32, tag="ot")
                    for kb in range(nkb):
                        rr = kb - 4 * c  # >= 0 on the diagonal chunk group
                        col0 = 0 if rr < 0 else 128 * rr
                        pst = psum_st.tile([128, 512], f32, tag="st")
                        nc.tensor.matmul(
                            pst[:, col0:512],
                            lhsT=kTr_sb[:, kb * 128:(kb + 1) * 128],
                            rhs=qTr[:, h, col0:512],
                            start=True,
                            stop=True,
                        )
                        if rr >= 0:
                            nc.vector.tensor_add(
                                pst[:, col0:col0 + 128],
                                pst[:, col0:col0 + 128],
                                dmask_sb,
                            )
                        e = epool.tile([128, 512], bf16, tag="E")
                        if col0 > 0:
                            nc.vector.memset(e[:, 0:col0], 0.0)
                        nc.scalar.activation(
                            out=e[:, col0:512], in_=pst[:, col0:512], func=EXP
                        )
                        nc.tensor.matmul(
                            psum_sum,
                            lhsT=onescol_sb,
                            rhs=e,
                            start=(kb == 0),
                            stop=(kb == nkb - 1),
                        )
                        nc.tensor.matmul(
                            psum_ot,
                            lhsT=v_sb[:, kb, :],
                            rhs=e,
                            start=(kb == 0),
                            stop=(kb == nkb - 1),
                        )
                    rsum = rsp.tile([1, 512], f32, tag="rsum")
                    rsumb = rsp.tile([1, 512], bf16, tag="rsumb")
                    nc.vector.reciprocal_approx_fast(out=rsum, in_=psum_sum)
                    nc.vector.tensor_copy(out=rsumb, in_=rsum)
                    pb = psum_pb.tile([128, 512], f32, tag="pb")
                    nc.tensor.matmul(
                        pb, lhsT=onesrow_sb, rhs=rsumb, start=True, stop=True
                    )
                    nc.scalar.copy(out=otc[:, h, :], in_=psum_ot)
                    nc.vector.tensor_mul(otc[:, h, :], otc[:, h, :], pb)

                # ---- output projection for this chunk's rows ----
                for tb in range(4):
                    row = tq0 + tb * 128
                    for oc in range(4):
                        po = psum.tile([128, 512], f32, tag="mm512")
                        for h in range(4):
                            nc.tensor.matmul(
                                po,
                                lhsT=otc[:, h, tb * 128:(tb + 1) * 128],
                                rhs=wc_sb[:, h, oc * 512:(oc + 1) * 512],
                                start=(h == 0),
                                stop=(h == 3),
                            )
                        osb = osbp.tile([128, 512], f32, tag="osb")
                        nc.scalar.copy(out=osb, in_=po)
                        nc.sync.dma_start(
                            out=out_d[row:row + 128, oc * 512:(oc + 1) * 512],
                            in_=osb,
                        )

    nc.compile()
    return nc


def _host_prep(x, Wq, Wkv, Wc):
    """Shard + relayout the full inputs into the 8 per-core input dicts."""
    import ml_dtypes

    bf = ml_dtypes.bfloat16
    dk, H, KV = DK, N_HEADS, N_KV_HEADS
    x = np.asarray(x, np.float32)
    Wq = np.asarray(Wq, np.float32)
    Wkv = np.asarray(Wkv, np.float32)
    Wc = np.asarray(Wc, np.float32)

    p = np.concatenate([np.arange(0, dk, 2), np.arange(1, dk, 2)])
    perm_q = np.concatenate([h * dk + p for h in range(H)])
    Wq_p = (Wq / math.sqrt(dk))[perm_q]
    perm_k = np.concatenate([g * dk + p for g in range(KV)])
    Wk_p = Wkv[:KV * dk][perm_k]
    Wv = Wkv[KV * dk:]

    pairs = np.arange(dk // 2, dtype=np.float64)
    freqs = 1.0 / (ROPE_THETA ** (2.0 * pairs / dk))
    ang = np.arange(S, dtype=np.float64)[:, None] * freqs[None, :]
    c2 = np.ascontiguousarray(np.cos(ang).astype(np.float32).T)  # [64, S]
    ss = np.ascontiguousarray(np.sin(ang).astype(np.float32).T)

    jj = np.arange(128)[None, :]
    pp = np.arange(128)[:, None]
    dmask = np.where(pp <= jj, 0.0, NEG).astype(np.float32)
    ident = np.eye(128, dtype=bf)
    onescol = np.ones((128, 1), bf)
    onesrow = np.ones((1, 128), bf)

    maps = []
    for core in range(NCORES):
        b, g = core // 4, core % 4
        wq_l = np.ascontiguousarray(
            Wq_p[512 * g:512 * g + 512].T.reshape(16, 128, 512).transpose(1, 0, 2)
        ).astype(bf)
        wkv_sl = np.concatenate(
            [Wk_p[g * dk:(g + 1) * dk], Wv[g * dk:(g + 1) * dk]], 0
        ).T  # [2048, 256]
        wkv_l = np.ascontiguousarray(
            wkv_sl.reshape(16, 128, 256).transpose(1, 0, 2)
        ).astype(bf)
        wc_l = np.ascontiguousarray(
            Wc[:, 512 * g:512 * g + 512].T.reshape(4, 128, 2048).transpose(1, 0, 2)
        ).astype(bf)
        maps.append(dict(
            x=np.ascontiguousarray(x[b]).astype(bf), wq=wq_l, wkv=wkv_l, wc=wc_l,
            cos2=c2, ss=ss, dmask=dmask, ident=ident,
            onescol=onescol, onesrow=onesrow,
        ))
    return maps


def kernel(x, Wq, Wkv, Wc):
    global _COMPILED, _LAST_RESULT
    from concourse.bass_utils import run_bass_kernel_spmd

    if _COMPILED is None:
        _COMPILED = _build()
    in_maps = _host_prep(x, Wq, Wkv, Wc)
    res = run_bass_kernel_spmd(
        _COMPILED, in_maps, core_ids=list(range(NCORES)), trace=_TRACE
    )
    _LAST_RESULT = res
    outs = [res.results[i]["out"] for i in range(NCORES)]
    full = np.stack(
        [outs[0] + outs[1] + outs[2] + outs[3],
         outs[4] + outs[5] + outs[6] + outs[7]], 0
    ).astype(np.float32)
    return full


# revision 9
# speedup vs baseline: 1.8315x; 1.0065x over previous
"""Causal self-attention (GQA + RoPE) Trainium2 Bass kernel, 8 NeuronCores.

Sharding: 2-way data parallel over batch x 4-way tensor parallel over heads.
Core c handles batch c//4 and query heads [4*(c%4), 4*(c%4)+4) plus the one
KV head g = c%4 that serves them (n_kv_heads=4 -> no KV replication).
Each core computes a partial [S, D] output (its heads' slice of the out
projection); the host sums the 4 partials per batch.

Device layouts are transposed ("feature-major"): x is transposed on-chip via
PE transposes; projections produce qT/kT/vT [dim, tokens]; attention scores
are computed as S^T = kT.T @ qT so softmax denominators come from a
ones-vector matmul (partition-dim sum) and the P@V contraction needs no
transposed probabilities.  RoPE is handled by de-interleaving the q/k weight
rows on the host so the rotation pairs become (p, p+64) partition pairs.
TensorEngine-facing tensors are bf16 (fp32 PSUM accumulation); softmax
tables/masks and the output stay fp32.
"""

import sys

if "/opt/trn_rl_repo" not in sys.path:
    sys.path.insert(0, "/opt/trn_rl_repo")

import math

import numpy as np

D_MODEL = 2048
N_HEADS = 16
N_KV_HEADS = 4
ROPE_THETA = 10000.0
B, S = 2, 2048
DK = D_MODEL // N_HEADS          # 128
NCORES = 8
NEG = -1e30

_COMPILED = None
_TRACE = False                   # test.py flips this for profiling runs
_LAST_RESULT = None              # BassKernelResults of the last run


def _build():
    import concourse.bacc as bacc
    import concourse.tile as tile
    from concourse import mybir

    f32 = mybir.dt.float32
    bf16 = mybir.dt.bfloat16

    nc = bacc.Bacc("TRN2", debug=False, target_bir_lowering=False)

    def inp(name, shape, dt=bf16):
        return nc.declare_dram_parameter(name, list(shape), dt, isOutput=False).ap()

    x_d = inp("x", [S, D_MODEL])
    wq_d = inp("wq", [128, 16, 512])
    wkv_d = inp("wkv", [128, 16, 256])
    wc_d = inp("wc", [128, 4, 2048])
    cos_d = inp("cos2", [64, S], f32)
    sin_d = inp("ss", [64, S], f32)
    dmask_d = inp("dmask", [128, 128], f32)
    ident_d = inp("ident", [128, 128])
    onescol_d = inp("onescol", [128, 1])
    onesrow_d = inp("onesrow", [1, 128])
    out_d = nc.declare_dram_parameter("out", [S, D_MODEL], f32, isOutput=True).ap()

    EXP = mybir.ActivationFunctionType.Exp
    LN = mybir.ActivationFunctionType.Ln

    with tile.TileContext(nc) as tc:
        with (
            tc.tile_pool(name="consts", bufs=1) as consts,
            tc.tile_pool(name="qpool", bufs=2) as qpool,
            tc.tile_pool(name="vch", bufs=2) as vchp,
            tc.tile_pool(name="tmp", bufs=2) as tmpp,
            tc.tile_pool(name="epool", bufs=6) as epool,
            tc.tile_pool(name="rsum", bufs=2) as rsp,
            tc.tile_pool(name="otp", bufs=2) as otp,
            tc.tile_pool(name="osb", bufs=3) as osbp,
            tc.tile_pool(name="psum", bufs=2, space="PSUM") as psum,
            tc.tile_pool(name="psum_st", bufs=2, space="PSUM") as psum_st,
            tc.tile_pool(name="psum_ot", bufs=2, space="PSUM") as psum_otp,
            tc.tile_pool(name="psum_acc", bufs=1, space="PSUM") as psum_acc,
            tc.tile_pool(name="psum_pb", bufs=1, space="PSUM") as psum_pb,
        ):
            # ---- constants / weights ----
            wq_sb = consts.tile([128, 16, 512], bf16, tag="wq")
            wkv_sb = consts.tile([128, 16, 256], bf16, tag="wkv")
            wc_sb = consts.tile([128, 4, 2048], bf16, tag="wc")
            c2_sb = consts.tile([64, S], f32, tag="cos2")
            ss_sb = consts.tile([64, S], f32, tag="ss")
            dmask_sb = consts.tile([128, 128], f32, tag="dmask")
            ident_sb = consts.tile([128, 128], bf16, tag="ident")
            onescol_sb = consts.tile([128, 1], bf16, tag="onescol")
            onesrow_sb = consts.tile([1, 128], bf16, tag="onesrow")
            kTr_sb = consts.tile([128, S], bf16, tag="kTr")
            v_sb = consts.tile([128, 16, 128], bf16, tag="V")
            xT = consts.tile([128, 16, S], bf16, tag="xT")

            nc.sync.dma_start_transpose(out=xT[:, 0, :], in_=x_d[:, 0:128])
            nc.sync.dma_start(out=wq_sb, in_=wq_d)
            nc.sync.dma_start(out=wkv_sb, in_=wkv_d)
            for db in range(1, 4):
                nc.sync.dma_start_transpose(
                    out=xT[:, db, :], in_=x_d[:, db * 128:(db + 1) * 128]
                )
            nc.sync.dma_start(out=c2_sb, in_=cos_d)
            nc.sync.dma_start(out=ss_sb, in_=sin_d)
            nc.sync.dma_start(out=dmask_sb, in_=dmask_d)
            nc.sync.dma_start(out=ident_sb, in_=ident_d)
            nc.sync.dma_start(out=onescol_sb, in_=onescol_d)
            nc.sync.dma_start(out=onesrow_sb, in_=onesrow_d)
            for db in range(4, 16):
                nc.sync.dma_start_transpose(
                    out=xT[:, db, :], in_=x_d[:, db * 128:(db + 1) * 128]
                )
            nc.sync.dma_start(out=wc_sb, in_=wc_d)

            def rope(dst, src, c):
                """dst[128,512] (bf16 SBUF) <- rotate(src[128,512] f32 PSUM).

                Row p<64 holds the even (te) element of pair p, row p+64 the
                odd (to): dst_lo = te*cos - to*sin; dst_hi = to*cos + te*sin.
                """
                cs = c2_sb[:, c * 512:(c + 1) * 512]
                sn = ss_sb[:, c * 512:(c + 1) * 512]
                t = tmpp.tile([128, 512], f32, tag="ropesin")
                t2 = tmpp.tile([128, 512], f32, tag="ropecos")
                nc.vector.tensor_mul(t[0:64, :], src[64:128, :], sn)
                nc.vector.tensor_mul(t[64:128, :], src[0:64, :], sn)
                nc.vector.tensor_mul(t2[0:64, :], src[0:64, :], cs)
                nc.vector.tensor_mul(t2[64:128, :], src[64:128, :], cs)
                nc.vector.tensor_sub(dst[0:64, :], t2[0:64, :], t[0:64, :])
                nc.vector.tensor_add(dst[64:128, :], t2[64:128, :], t[64:128, :])

            for c in range(4):  # 512-token chunks
                tq0 = c * 512
                # ---- Q projection + rope ----
                qTr = qpool.tile([128, 4, 512], bf16, tag="qTr")
                for m in range(4):
                    pq = psum.tile([128, 512], f32, tag="mm512")
                    for db in range(16):
                        nc.tensor.matmul(
                            pq,
                            lhsT=wq_sb[:, db, m * 128:(m + 1) * 128],
                            rhs=xT[:, db, tq0:tq0 + 512],
                            start=(db == 0),
                            stop=(db == 15),
                        )
                    rope(qTr[:, m, :], pq, c)

                # ---- K projection + rope ----
                pk = psum.tile([128, 512], f32, tag="mm512")
                for db in range(16):
                    nc.tensor.matmul(
                        pk,
                        lhsT=wkv_sb[:, db, 0:128],
                        rhs=xT[:, db, tq0:tq0 + 512],
                        start=(db == 0),
                        stop=(db == 15),
                    )
                rope(kTr_sb[:, tq0:tq0 + 512], pk, c)

                # ---- V projection; transpose to [tk, hd] tiles ----
                pv = psum.tile([128, 512], f32, tag="mm512")
                for db in range(16):
                    nc.tensor.matmul(
                        pv,
                        lhsT=wkv_sb[:, db, 128:256],
                        rhs=xT[:, db, tq0:tq0 + 512],
                        start=(db == 0),
                        stop=(db == 15),
                    )
                vch = vchp.tile([128, 512], bf16, tag="vch")
                nc.scalar.copy(out=vch, in_=pv)
                for rr in range(4):
                    nc.scalar.dma_start_transpose(
                        out=v_sb[:, 4 * c + rr, :],
                        in_=vch[:, rr * 128:(rr + 1) * 128],
                    )

                # ---- attention for tq chunk c, all 4 heads ----
                nkb = 4 * c + 4
                otc = otp.tile([128, 4, 512], bf16, tag="OT")
                for h in range(4):
                    psum_sum = psum_acc.tile([1, 512], f32, tag="sums")
                    psum_ot = psum_otp.tile([128, 512], f32, tag="ot")
                    for kb in range(nkb):
                        rr = kb - 4 * c  # >= 0 on the diagonal chunk group
                        col0 = 0 if rr < 0 else 128 * rr
                        pst = psum_st.tile([128, 512], f32, tag="st")
                        nc.tensor.matmul(
                            pst[:, col0:512],
                            lhsT=kTr_sb[:, kb * 128:(kb + 1) * 128],
                            rhs=qTr[:, h, col0:512],
                            start=True,
                            stop=True,
                        )
                        if rr >= 0:
                            nc.vector.tensor_add(
                                pst[:, col0:col0 + 128],
                                pst[:, col0:col0 + 128],
                                dmask_sb,
                            )
                        e = epool.tile([128, 512], bf16, tag="E")
                        if col0 > 0:
                            nc.vector.memset(e[:, 0:col0], 0.0)
                        nc.scalar.activation(
                            out=e[:, col0:512], in_=pst[:, col0:512], func=EXP
                        )
                        nc.tensor.matmul(
                            psum_sum,
                            lhsT=onescol_sb,
                            rhs=e,
                            start=(kb == 0),
                            stop=(kb == nkb - 1),
                        )
                        nc.tensor.matmul(
                            psum_ot,
                            lhsT=v_sb[:, kb, :],
                            rhs=e,
                            start=(kb == 0),
                            stop=(kb == nkb - 1),
                        )
                    rsum = rsp.tile([1, 512], f32, tag="rsum")
                    rsumb = rsp.tile([1, 512], bf16, tag="rsumb")
                    nc.vector.reciprocal_approx_fast(out=rsum, in_=psum_sum)
                    nc.vector.tensor_copy(out=rsumb, in_=rsum)
                    pb = psum_pb.tile([128, 512], f32, tag="pb")
                    nc.tensor.matmul(
                        pb, lhsT=onesrow_sb, rhs=rsumb, start=True, stop=True
                    )
                    nc.scalar.copy(out=otc[:, h, :], in_=psum_ot)
                    nc.vector.tensor_mul(otc[:, h, :], otc[:, h, :], pb)

                # ---- output projection for this chunk's rows ----
                for tb in range(4):
                    row = tq0 + tb * 128
                    for oc in range(4):
                        po = psum.tile([128, 512], f32, tag="mm512")
                        for h in range(4):
                            nc.tensor.matmul(
                                po,
                                lhsT=otc[:, h, tb * 128:(tb + 1) * 128],
                                rhs=wc_sb[:, h, oc * 512:(oc + 1) * 512],
                                start=(h == 0),
                                stop=(h == 3),
                            )
                        osb = osbp.tile([128, 512], f32, tag="osb")
                        nc.scalar.copy(out=osb, in_=po)
                        nc.sync.dma_start(
                            out=out_d[row:row + 128, oc * 512:(oc + 1) * 512],
                            in_=osb,
                        )

    nc.compile()
    return nc


def _host_prep(x, Wq, Wkv, Wc):
    """Shard + relayout the full inputs into the 8 per-core input dicts."""
    import ml_dtypes

    bf = ml_dtypes.bfloat16
    dk, H, KV = DK, N_HEADS, N_KV_HEADS
    x = np.asarray(x, np.float32)
    Wq = np.asarray(Wq, np.float32)
    Wkv = np.asarray(Wkv, np.float32)
    Wc = np.asarray(Wc, np.float32)

    p = np.concatenate([np.arange(0, dk, 2), np.arange(1, dk, 2)])
    perm_q = np.concatenate([h * dk + p for h in range(H)])
    Wq_p = (Wq / math.sqrt(dk))[perm_q]
    perm_k = np.concatenate([g * dk + p for g in range(KV)])
    Wk_p = Wkv[:KV * dk][perm_k]
    Wv = Wkv[KV * dk:]

    pairs = np.arange(dk // 2, dtype=np.float64)
    freqs = 1.0 / (ROPE_THETA ** (2.0 * pairs / dk))
    ang = np.arange(S, dtype=np.float64)[:, None] * freqs[None, :]
    c2 = np.ascontiguousarray(np.cos(ang).astype(np.float32).T)  # [64, S]
    ss = np.ascontiguousarray(np.sin(ang).astype(np.float32).T)

    jj = np.arange(128)[None, :]
    pp = np.arange(128)[:, None]
    dmask = np.where(pp <= jj, 0.0, NEG).astype(np.float32)
    ident = np.eye(128, dtype=bf)
    onescol = np.ones((128, 1), bf)
    onesrow = np.ones((1, 128), bf)

    maps = []
    for core in range(NCORES):
        b, g = core // 4, core % 4
        wq_l = np.ascontiguousarray(
            Wq_p[512 * g:512 * g + 512].T.reshape(16, 128, 512).transpose(1, 0, 2)
        ).astype(bf)
        wkv_sl = np.concatenate(
            [Wk_p[g * dk:(g + 1) * dk], Wv[g * dk:(g + 1) * dk]], 0
        ).T  # [2048, 256]
        wkv_l = np.ascontiguousarray(
            wkv_sl.reshape(16, 128, 256).transpose(1, 0, 2)
        ).astype(bf)
        wc_l = np.ascontiguousarray(
            Wc[:, 512 * g:512 * g + 512].T.reshape(4, 128, 2048).transpose(1, 0, 2)
        ).astype(bf)
        maps.append(dict(
            x=np.ascontiguousarray(x[b]).astype(bf), wq=wq_l, wkv=wkv_l, wc=wc_l,
            cos2=c2, ss=ss, dmask=dmask, ident=ident,
            onescol=onescol, onesrow=onesrow,
        ))
    return maps


def kernel(x, Wq, Wkv, Wc):
    global _COMPILED, _LAST_RESULT
    from concourse.bass_utils import run_bass_kernel_spmd

    if _COMPILED is None:
        _COMPILED = _build()
    in_maps = _host_prep(x, Wq, Wkv, Wc)
    res = run_bass_kernel_spmd(
        _COMPILED, in_maps, core_ids=list(range(NCORES)), trace=_TRACE
    )
    _LAST_RESULT = res
    outs = [res.results[i]["out"] for i in range(NCORES)]
    full = np.stack(
        [outs[0] + outs[1] + outs[2] + outs[3],
         outs[4] + outs[5] + outs[6] + outs[7]], 0
    ).astype(np.float32)
    return full
